# revision 14
# baseline (speedup 1.0000x reference)
# Bass/Trainium2 kernel for nn_AA2_Module_75359496175785 (sparse_attention).
#
# Math (per batch item b; x: (C,N) with C=128, N=H*W=16384):
#   q  = Wq x + bq;  k_g = Wk_g pool(x) + bk_g   (pooling commutes with 1x1 conv)
#   e_g = q^T k_g;   a_g = softmax(alpha_g e_g, axis=keys)
#   out = gamma0 k_0 a_0^T + x + gamma1 k_1 a_1^T
#
# Structure:
#  * alpha/256 folded into Wk host-side; gamma/alpha folded into the kT scale.
#  * M = Wq^T k_cat expanded host-side: M = (Wks^T Wq)^T pool(x) + mb, so the
#    energy weights come straight from pooled x - a ~1.5us tail after the last
#    input byte instead of a serial q/k chain.
#  * Phase 0 streams x in 16 pieces of 1024 cols; per piece: ACT casts to bf16,
#    DVE pool-reduces (half-sums, added at the end), and a 128-col blip matmul
#    keeps the PE HAM window busy.  A ~5us burn of back-to-back matmuls right
#    after the weights land lifts the PE clock gate to 2.4 GHz before phase 1.
#  * Phase 1: software-pipelined groups of 1024 columns with a deep skew so the
#    exp->sum->recip->attn chain never blocks the PE:
#      PE : energy(g) | sums(g-1) | out(g-3)+x-add(idb, half)
#      ACT: exp(g)    | psum->sbuf copy(g-3) (half)
#      DVE: recip(g-1), attn-mul tail(g-1), stt copy+x-add(g-3) (half)
#      GPS: attn-mul head(g-1)
import numpy as np

B, C, H, W = 8, 128, 128, 128
N = H * W
PP = 8
NKEYS = 64
PIECE = 1024      # phase-0 dma piece = 8 rows of H (half a pool-block row)
NPIECE = N // PIECE
GRP = 512
PAIR = 2 * GRP    # phase-1 iteration width
NPAIR = N // PAIR
TT_G = 896        # columns (of each 1024) whose attn-multiply runs on gpsimd
NWARM = 26        # back-to-back matmuls per warm burn of the PE clock gate
SKEW = 2          # phase-1 pipeline depth between energy(g) and out(g-SKEW)

_CACHE = {}


def _build_nc():
    import concourse.bass as bass  # noqa: F401
    from concourse import bacc, mybir
    import concourse.tile as tile

    f32 = mybir.dt.float32
    bf16 = mybir.dt.bfloat16
    AF = mybir.ActivationFunctionType

    nc = bacc.Bacc(None, target_bir_lowering=False)

    x_d = nc.dram_tensor("x", [C, N], f32, kind="ExternalInput")
    # bf16 weights: [WmT0 | WmT1 | WksT0 | WksT1 | idb | ones_bd]
    wb_d = nc.dram_tensor("wb", [C, 6 * C], bf16, kind="ExternalInput")
    # f32 weights: [bks0 bks1 bq gvec]
    wf_d = nc.dram_tensor("wf", [C, 4], f32, kind="ExternalInput")
    out_d = nc.dram_tensor("out", [C, N], f32, kind="ExternalOutput")

    with tile.TileContext(nc) as tc:
        with (
            tc.tile_pool(name="const", bufs=1) as const,
            tc.tile_pool(name="big", bufs=1) as big,
            tc.tile_pool(name="expp", bufs=5) as expp,
            tc.tile_pool(name="rp", bufs=4) as rp,
            tc.tile_pool(name="attnp", bufs=5) as attnp,
            tc.tile_pool(name="outp", bufs=5) as outp,
        ):
            wb = const.tile([C, 6 * C], bf16)
            wf = const.tile([C, 4], f32)
            nc.sync.dma_start(wb[:], wb_d[:])
            nc.sync.dma_start(wf[:], wf_d[:])
            wmT0 = wb[:, 0:C]
            wmT1 = wb[:, C:2 * C]
            wksT0 = wb[:, 2 * C:3 * C]
            wksT1 = wb[:, 3 * C:4 * C]
            idb = wb[:, 4 * C:5 * C]
            ones_bd = wb[:, 5 * C:6 * C]
            bks0 = wf[:, 0:1]
            bks1 = wf[:, 1:2]
            bq = wf[:, 2:3]
            gvec = wf[:, 3:4]

            x_sb = big.tile([C, N], f32)
            x_bf = big.tile([C, N], bf16)
            xpa = big.tile([C, NKEYS], f32)
            xpb = big.tile([C, NKEYS], f32)
            xp_bf = big.tile([C, NKEYS], bf16)
            m_bf = big.tile([C, C], bf16)
            k_bf = big.tile([C, C], bf16)
            kT = big.tile([C, C], bf16)
            ebias = big.tile([C, 1], f32)
            bq_bf = big.tile([C, 1], bf16)
            tiny = big.tile([C, 1], f32)

            # ---- phase 0: stream x, cast + pool per piece, keep PE warm ----
            ph0 = tc.tile_pool(name="ps0", bufs=1, space="PSUM")
            ps0 = ph0.__enter__()
            scr_ps = ps0.tile([C, C], f32, tag="scrp")

            # preload the exp table set + prime small constants
            nc.scalar.activation(tiny[:], wf[:, 0:1], AF.Exp)
            nc.vector.tensor_copy(bq_bf[:], bq)
            for p in range(NPIECE):
                psl = bass.ts(p, PIECE)
                nc.sync.dma_start(x_sb[:, psl], x_d[:, psl])
                nc.scalar.copy(
                    x_bf[:, bass.ds(p * PIECE, 768)],
                    x_sb[:, bass.ds(p * PIECE, 768)],
                )
                nc.gpsimd.tensor_copy(
                    x_bf[:, bass.ds(p * PIECE + 768, 256)],
                    x_sb[:, bass.ds(p * PIECE + 768, 256)],
                )
                xc = x_sb[:, psl].rearrange(
                    "p (h pj w) -> p pj h w", h=PP, pj=PP, w=16
                )
                xpdst = xpa if p % 2 == 0 else xpb
                nc.vector.tensor_reduce(
                    xpdst[:, (p // 2) * PP:(p // 2 + 1) * PP], xc,
                    axis=mybir.AxisListType.XY, op=mybir.AluOpType.add,
                )
                if p in (11, 13):
                    # warm the PE clock gate right before phase 1 (two burns
                    # while the tail pieces stream in)
                    for _ in range(NWARM):
                        nc.tensor.matmul(
                            scr_ps[:], ones_bd, x_bf[:, bass.ds(p * PIECE, C)],
                            start=True, stop=True,
                        )

            # ---- mid: M (energy weights), keys, kT, ebias ----
            m_ps = ps0.tile([C, C], f32, tag="mps")
            kk_ps = ps0.tile([C, C], f32, tag="kkps")
            kT_ps = ps0.tile([C, C], bf16, tag="ktps")
            eb_ps = ps0.tile([C, 1], f32, tag="ebps")

            nc.vector.tensor_tensor(
                out=xp_bf[:], in0=xpa[:], in1=xpb[:], op=mybir.AluOpType.add
            )
            nc.tensor.matmul(m_ps[:, 0:NKEYS], wmT0, xp_bf[:], start=True, stop=True)
            nc.tensor.matmul(m_ps[:, NKEYS:], wmT1, xp_bf[:], start=True, stop=True)
            # NOTE: the M bias (Wq^T bks) is a per-column constant within each
            # branch's key group, so softmax cancels it - no bias needed here.
            nc.scalar.copy(m_bf[:], m_ps[:])
            nc.tensor.matmul(kk_ps[:, 0:NKEYS], wksT0, xp_bf[:], start=True, stop=True)
            nc.tensor.matmul(kk_ps[:, NKEYS:], wksT1, xp_bf[:], start=True, stop=True)
            nc.scalar.activation(
                k_bf[:, 0:NKEYS], kk_ps[:, 0:NKEYS], AF.Identity, bias=bks0, scale=1.0
            )
            nc.scalar.activation(
                k_bf[:, NKEYS:], kk_ps[:, NKEYS:], AF.Identity, bias=bks1, scale=1.0
            )
            nc.tensor.matmul(eb_ps[:], k_bf[:], bq_bf[:], start=True, stop=True)
            nc.tensor.transpose(kT_ps[:], k_bf[:], idb)
            nc.vector.tensor_copy(ebias[:], eb_ps[:])
            nc.scalar.activation(kT[:], kT_ps[:], AF.Copy, scale=gvec)
            ph0.__exit__(None, None, None)

            # ---- phase 1: deep-skewed pipeline over 1024-column groups ----
            ph_e = tc.tile_pool(name="ps_e", bufs=2, space="PSUM")
            ps_e = ph_e.__enter__()
            ph_s = tc.tile_pool(name="ps_s", bufs=1, space="PSUM")
            ps_s = ph_s.__enter__()
            ph_u = tc.tile_pool(name="ps_u", bufs=1, space="PSUM")
            ps_u = ph_u.__enter__()

            exps = [None] * NPAIR
            attns = [None] * NPAIR
            for g in range(NPAIR + SKEW):
                ga, gb, gc = g, g - 1, g - SKEW
                # oldest stage first so no engine queue ever waits behind a
                # younger, not-yet-ready instruction (head-of-line blocking)
                if 0 <= gc < NPAIR:
                    c0 = bass.ds(gc * PAIR, GRP)
                    c1 = bass.ds(gc * PAIR + GRP, GRP)
                    u0 = ps_u.tile([C, GRP], f32, tag="uh0")
                    u1 = ps_u.tile([C, GRP], f32, tag="uh1")
                    nc.tensor.matmul(
                        u0[:], kT[:], attns[gc][0][:], start=True, stop=False
                    )
                    nc.tensor.matmul(
                        u1[:], kT[:], attns[gc][1][:], start=True, stop=True
                    )
                    nc.tensor.matmul(
                        u0[:], idb, x_bf[:, c0], start=False, stop=True
                    )
                    o0 = outp.tile([C, GRP], f32, name="o0", tag="o0")
                    o1 = outp.tile([C, GRP], f32, name="o1", tag="o1")
                    nc.scalar.activation(o0[:], u0[:], AF.Copy)
                    nc.sync.dma_start(out_d[:, c0], o0[:])
                    nc.vector.scalar_tensor_tensor(
                        out=o1[:],
                        in0=u1[:],
                        scalar=1.0,
                        in1=x_sb[:, c1],
                        op0=mybir.AluOpType.mult,
                        op1=mybir.AluOpType.add,
                    )
                    nc.sync.dma_start(out_d[:, c1], o1[:])
                if 0 <= gb < NPAIR:
                    s_ps = ps_s.tile([C, PAIR], f32, tag="sps")
                    nc.tensor.matmul(
                        s_ps[:, 0:GRP], ones_bd, exps[gb][:, 0:GRP],
                        start=True, stop=True,
                    )
                    nc.tensor.matmul(
                        s_ps[:, GRP:], ones_bd, exps[gb][:, GRP:],
                        start=True, stop=True,
                    )
                    # separate half tiles throughout: co-written tiles would
                    # WAW-serialize engine queues (Tile tracks whole tiles)
                    r0 = rp.tile([C, GRP], f32, name="r0", tag="r0")
                    r1 = rp.tile([C, GRP], f32, name="r1", tag="r1")
                    nc.vector.reciprocal_approx_fast(out=r0[:], in_=s_ps[:, 0:GRP])
                    nc.vector.reciprocal_approx_fast(out=r1[:], in_=s_ps[:, GRP:])
                    at0 = attnp.tile([C, GRP], bf16, name="attn0", tag="at0")
                    at1 = attnp.tile([C, GRP], bf16, name="attn1", tag="at1")
                    nc.gpsimd.tensor_mul(at0[:], exps[gb][:, 0:GRP], r0[:])
                    nc.gpsimd.tensor_mul(at1[:], exps[gb][:, GRP:], r1[:])
                    attns[gb] = (at0, at1)
                if ga < NPAIR:
                    a0 = bass.ds(ga * PAIR, GRP)
                    a1 = bass.ds(ga * PAIR + GRP, GRP)
                    e_ps = ps_e.tile([C, PAIR], f32, tag="eps")
                    nc.tensor.matmul(
                        e_ps[:, 0:GRP], m_bf[:], x_bf[:, a0], start=True, stop=True
                    )
                    nc.tensor.matmul(
                        e_ps[:, GRP:], m_bf[:], x_bf[:, a1], start=True, stop=True
                    )
                    exps[ga] = expp.tile([C, PAIR], bf16, name="exp_sb")
                    nc.scalar.activation(
                        exps[ga][:], e_ps[:], AF.Exp, bias=ebias[:, 0:1], scale=1.0
                    )
            ph_u.__exit__(None, None, None)
            ph_s.__exit__(None, None, None)
            ph_e.__exit__(None, None, None)

    nc.compile()
    return nc


def _get_nc():
    if "nc" not in _CACHE:
        _CACHE["nc"] = _build_nc()
    return _CACHE["nc"]


def _make_in_maps(x, Wq, bq, Wk, bk, Wk1, bk1, gamma, gamma1, aphal, aphal1):
    a0 = float(np.asarray(aphal).reshape(-1)[0])
    a1 = float(np.asarray(aphal1).reshape(-1)[0])
    g0 = float(np.asarray(gamma).reshape(-1)[0])
    g1 = float(np.asarray(gamma1).reshape(-1)[0])

    f = np.float32
    Wq = np.asarray(Wq, f)
    Wks0 = np.asarray(Wk, f) * (a0 / 256.0)
    Wks1 = np.asarray(Wk1, f) * (a1 / 256.0)
    bks0 = np.asarray(bk, f).reshape(C) * a0
    bks1 = np.asarray(bk1, f).reshape(C) * a1
    wmT0 = Wks0.T @ Wq           # stationary for M = (Wq^T Wks) pool(x)
    wmT1 = Wks1.T @ Wq
    eye = np.eye(C, dtype=f)
    ones_bd = np.kron(np.eye(2, dtype=f), np.ones((NKEYS, NKEYS), f))
    wb = np.concatenate(
        [wmT0, wmT1, Wks0.T, Wks1.T, eye, ones_bd], axis=1
    ).astype("bfloat16")
    gvec = np.concatenate(
        [np.full((NKEYS, 1), g0 / a0, f), np.full((NKEYS, 1), g1 / a1, f)]
    )
    wf = np.concatenate(
        [
            bks0.reshape(C, 1), bks1.reshape(C, 1),
            np.asarray(bq, f).reshape(C, 1),
            gvec,
        ],
        axis=1,
    ).astype(f)
    wb = np.ascontiguousarray(wb)
    wf = np.ascontiguousarray(wf)
    in_maps = []
    for b in range(B):
        in_maps.append({
            "x": np.ascontiguousarray(np.asarray(x)[b].reshape(C, N), dtype=f),
            "wb": wb,
            "wf": wf,
        })
    return in_maps


def kernel(x, Wq, bq, Wk, bk, Wk1, bk1, gamma, gamma1, aphal, aphal1, **_):
    import ml_dtypes  # noqa: F401
    from concourse.bass_utils import run_bass_kernel_spmd

    nc = _get_nc()
    in_maps = _make_in_maps(
        np.asarray(x), np.asarray(Wq), np.asarray(bq), np.asarray(Wk),
        np.asarray(bk), np.asarray(Wk1), np.asarray(bk1), np.asarray(gamma),
        np.asarray(gamma1), np.asarray(aphal), np.asarray(aphal1),
    )
    res = None
    last_exc = None
    for _attempt in range(3):
        try:
            res = run_bass_kernel_spmd(nc, in_maps, core_ids=list(range(B)))
            break
        except Exception as e:  # transient NRT_EXEC_UNIT_UNRECOVERABLE faults
            last_exc = e
            import time as _time
            _time.sleep(2.0)
    if res is None:
        raise last_exc
    out = np.stack([res.results[b]["out"].reshape(C, H, W) for b in range(B)])
    return out.astype(np.float32)
